# revision 16
# baseline (speedup 1.0000x reference)
"""Multi-head attention (B=4, N=2048, D=1024, H=16) on 8 Trainium2 NeuronCores.

Sharding: core c = 2*b + hg handles batch b and head-group hg (8 of 16 heads).
Host pre-transposes x and mask per batch, slices Wq/Wk/Wv columns and Wo rows
per head group, and sums the two partial outputs per batch (+ bo) at the end.

v2 design (vs baseline): engine-balanced software pipeline tuned to keep the
PE continuously busy (p-state ramp: the tensor engine only reaches 2.4 GHz
after ~3us of uninterrupted work; any idle gap drops it to 1.2 GHz).

  Phase P: k/v projections for all seq + q projection for the first q-chunk
    only. q-proj for q-chunks 1..3 is deferred and used as PE filler work
    during the attention phase (which is ACT/exp-bound).
  Attention, per q-chunk (512 q) / head-pair / k-chunk (128 k):
    scores (PE, 2 matmuls K=64) -> in-place mask multiply (DVE or GpSimd,
    round-robin) -> exp (ACT, fp32 psum -> fp16 sbuf) -> ctx accumulate
    (PE, K=128, M=65: row 64 = softmax denominator via ones column of v).
  Normalize per head-pair: DVE reciprocal on the denominator row, GpSimd
    partition_broadcast across 64 partitions, DVE/GpSimd multiplies, PE
    ident-shift to pack head-pair ctx into one 128-row fp16 tile.
  Output projection per q-chunk is also deferred into the next q-chunk's
    unit stream as PE filler; result DMA'd straight from PSUM to DRAM.

PSUM budget (8 banks): scores ring 2x[128,2,512] (4) + ctx accum ring
2x[65,2,512] (4). Shift/out-proj/q-proj psum allocations share the scores
ring. Mask is fp16, duplicated per head-pair-half by DMA so the multiply is
a single [128,2,512] op per k-chunk.
"""
import os
from contextlib import ExitStack

import numpy as np

from concourse import bacc, mybir, tile
from concourse import bass_utils

P = 128
NSEQ = 2048          # sequence length
DMODEL = 1024        # model dim
HD = 512             # per-core head dim total (8 heads x 64)
NH = 8               # heads per core
DH = 64              # head depth
DC = DMODEL // P     # 8 d_model chunks
NQC = 4              # q chunks of 512
F32 = mybir.dt.float32
F32R = mybir.dt.float32r
F16 = mybir.dt.float16
AF = mybir.ActivationFunctionType
OP = mybir.AluOpType

N_WARM = 12          # PE clock warm-up matmuls
FILLER_EVERY = 4     # emit one filler element every this many units

_CACHE: dict = {}
LAST_RESULTS = None


def _patch_act_tables():
    """Force every activation onto the one table set containing
    exp+ln+copy+identity, so the kernel performs a single ACT_TABLE_LOAD
    instead of thrashing between per-function sets (1.3us each)."""
    import functools
    from concourse import bacc as _bacc
    from concourse import hw_specs as _hw
    if getattr(_bacc, "_act_tables_patched", False):
        return
    orig = _hw.get_activation_tables

    @functools.cache
    def patched(arch):
        tabs = dict(orig(arch))
        full = "natural_log_exp_and_others"
        keep = tabs[full]
        strip = {f for f in keep}
        out = {}
        for name, funcs in tabs.items():
            out[name] = funcs if name == full else (funcs - strip)
        return out

    _bacc.get_activation_tables = patched
    _bacc._act_tables_patched = True


def _build():
    _patch_act_tables()
    nc = bacc.Bacc("TRN2", target_bir_lowering=False, debug=False,
                   enable_asserts=False, num_devices=8)

    xT = nc.dram_tensor("xT", [DMODEL, NSEQ], F16, kind="ExternalInput").ap()
    maskT = nc.dram_tensor("maskT", [NSEQ, NSEQ], F16, kind="ExternalInput").ap()
    wq_d = nc.dram_tensor("wq", [DMODEL, HD], F16, kind="ExternalInput").ap()
    wk_d = nc.dram_tensor("wk", [DMODEL, HD], F16, kind="ExternalInput").ap()
    wv_d = nc.dram_tensor("wv", [DMODEL, HD], F16, kind="ExternalInput").ap()
    wo_d = nc.dram_tensor("wo", [P, 4, DMODEL], F16, kind="ExternalInput").ap()
    id_d = nc.dram_tensor("ident", [DH, P], F16, kind="ExternalInput").ap()
    bq_d = nc.dram_tensor("bq2", [P, 4], F32, kind="ExternalInput").ap()
    bk_d = nc.dram_tensor("bk2", [P, 4], F32, kind="ExternalInput").ap()
    bvr_d = nc.dram_tensor("bvr", [P, HD], F32, kind="ExternalInput").ap()
    ones_d = nc.dram_tensor("ones2", [P, 512], F32R, kind="ExternalInput").ap()
    y_d = nc.dram_tensor("y", [NSEQ, DMODEL], F16, kind="ExternalOutput").ap()

    xT_r = xT.rearrange("(dc p) n -> p dc n", p=P)        # [128, 8, 2048]
    maskT_r = maskT.rearrange("(kc p) q -> p kc q", p=P)  # [128, 16, 2048]

    with tile.TileContext(nc) as tc, ExitStack() as ctx:
        persist = ctx.enter_context(tc.tile_pool(name="persist", bufs=1))
        qT = persist.tile([P, 4, NSEQ], F16)     # [hd%128, hd-chunk, seq]
        kT = persist.tile([P, 4, NSEQ], F16)
        v = persist.tile([P, 16, NH, DH + 1], F16)  # [seq%128, seq-chunk, h, d|1]
        wq = persist.tile([P, DC, HD], F16)
        wk = persist.tile([P, DC, HD], F16)
        wv = persist.tile([P, DC, HD], F16)
        wo = persist.tile([P, 4, DMODEL], F16)
        ident = persist.tile([DH, P], F16)
        ones = persist.tile([P, 512], F32R)
        bqs = persist.tile([P, 4], F32)
        bks = persist.tile([P, 4], F32)
        bvr = persist.tile([P, HD], F32)

        mqp = ctx.enter_context(tc.tile_pool(name="mq", bufs=2))
        exp_pool = ctx.enter_context(tc.tile_pool(name="ex", bufs=4))
        ctxp = ctx.enter_context(tc.tile_pool(name="cpr", bufs=8))
        rtp = ctx.enter_context(tc.tile_pool(name="rt", bufs=1))
        rbp = ctx.enter_context(tc.tile_pool(name="rb", bufs=1))
        cxop = ctx.enter_context(tc.tile_pool(name="cxo", bufs=2))
        pyp = ctx.enter_context(tc.tile_pool(name="py_sb", bufs=2))
        xwp = ctx.enter_context(tc.tile_pool(name="xw", bufs=2))

        def load_x_window(n, name):
            xw = xwp.tile([P, DC, 512], F16, tag="xw", name=name)
            for dc in range(DC):
                nc.sync.dma_start(out=xw[:, dc, :],
                                  in_=xT_r[:, dc, n * 512:(n + 1) * 512])
            return xw

        # ---- initial DMAs (queue order matters: earliest-needed first) ----
        nc.sync.dma_start(out=ones, in_=ones_d)
        for dc in range(DC):
            nc.sync.dma_start(out=wk[:, dc, :],
                              in_=wk_d.rearrange("(dc p) m -> p dc m", p=P)[:, dc, :])
        for dc in range(DC):
            nc.sync.dma_start(out=wv[:, dc, :],
                              in_=wv_d.rearrange("(dc p) m -> p dc m", p=P)[:, dc, :])
        for dc in range(DC):
            nc.sync.dma_start(out=wq[:, dc, :],
                              in_=wq_d.rearrange("(dc p) m -> p dc m", p=P)[:, dc, :])
        nc.sync.dma_start(out=bqs, in_=bq_d)
        nc.sync.dma_start(out=bks, in_=bk_d)
        nc.sync.dma_start(out=bvr, in_=bvr_d)
        nc.sync.dma_start(out=ident, in_=id_d)
        nc.sync.dma_start(out=wo, in_=wo_d)

        nc.vector.tensor_copy(
            out=v[:, :, :, DH],
            in_=ones[:, 0:P].rearrange("p (a b) -> p a b", b=NH))

        def prefetch_mask(qc):
            mq = mqp.tile([P, 16, 2, 512], F16, tag="mq", name=f"mq{qc}")
            for kc in range(16):
                for hp in range(2):
                    nc.sync.dma_start(
                        out=mq[:, kc, hp, :],
                        in_=maskT_r[:, kc, qc * 512:(qc + 1) * 512])
            return mq

        # ---------------- phase P: projections ----------------
        with tc.tile_pool(name="pp", bufs=6, space="PSUM") as pp:
            wt = pp.tile([P, 512], F32, tag="pp", name="warm")
            for i in range(N_WARM):
                nc.tensor.matmul(wt, lhsT=ones[:, 0:P], rhs=ones,
                                 start=(i == 0), stop=(i == N_WARM - 1))
            mq_tiles = {0: prefetch_mask(0)}
            for n in range(NQC):
                x_n = load_x_window(n, f"xn{n}")
                for m in range(4):
                    ps = pp.tile([P, 512], F32, tag="pp", name="psk")
                    for dc in range(DC):
                        nc.tensor.matmul(ps,
                                         lhsT=wk[:, dc, m * 128:(m + 1) * 128],
                                         rhs=x_n[:, dc, :],
                                         start=(dc == 0), stop=(dc == DC - 1))
                    nc.scalar.activation(
                        out=kT[:, m, n * 512:(n + 1) * 512], in_=ps,
                        func=AF.Identity, bias=bks[:, m:m + 1], scale=1.0)
                for s4 in range(4):
                    psv = pp.tile([P, 512], F32, tag="pp", name="psv")
                    for dc in range(DC):
                        nc.tensor.matmul(psv,
                                         lhsT=x_n[:, dc, s4 * 128:(s4 + 1) * 128],
                                         rhs=wv[:, dc, :],
                                         start=(dc == 0), stop=(dc == DC - 1))
                    s = n * 4 + s4
                    nc.vector.tensor_tensor(
                        v[:, s, :, 0:DH],
                        psv.rearrange("p (h d) -> p h d", d=DH),
                        bvr.rearrange("p (h d) -> p h d", d=DH),
                        OP.add)
                if n == 0:
                    for m in range(4):
                        psq = pp.tile([P, 512], F32, tag="pp", name="psq0")
                        for dc in range(DC):
                            nc.tensor.matmul(psq,
                                             lhsT=wq[:, dc, m * 128:(m + 1) * 128],
                                             rhs=x_n[:, dc, :],
                                             start=(dc == 0), stop=(dc == DC - 1))
                        nc.scalar.activation(
                            out=qT[:, m, 0:512], in_=psq,
                            func=AF.Identity, bias=bqs[:, m:m + 1], scale=1.0)

        # ---------------- attention + output projection ----------------
        ctx_tiles = {qc: [] for qc in range(NQC)}
        with tc.tile_pool(name="aps", bufs=1, space="PSUM") as aps:

            def qproj_fillers(qc, xq):
                """Deferred q projection for q-chunk qc: 8 elements of 4 mm."""
                els = []
                for m in range(4):
                    state = {}

                    def el1(m=m, state=state, xq=xq):
                        psq = aps.tile([P, 2, 512], F32, tag="ss",
                                       bufs=2, name=f"psq{qc}_{m}")
                        state["psq"] = psq
                        for dc in range(4):
                            nc.tensor.matmul(
                                psq[:, 0, :],
                                lhsT=wq[:, dc, m * 128:(m + 1) * 128],
                                rhs=xq[:, dc, :],
                                start=(dc == 0), stop=False)

                    def el2(m=m, state=state, qc=qc, xq=xq):
                        psq = state["psq"]
                        for dc in range(4, DC):
                            nc.tensor.matmul(
                                psq[:, 0, :],
                                lhsT=wq[:, dc, m * 128:(m + 1) * 128],
                                rhs=xq[:, dc, :],
                                start=False, stop=(dc == DC - 1))
                        nc.scalar.activation(
                            out=qT[:, m, qc * 512:(qc + 1) * 512],
                            in_=psq[:, 0, :],
                            func=AF.Identity, bias=bqs[:, m:m + 1], scale=1.0)

                    els += [el1, el2]
                return els

            def outproj_fillers(qc):
                """Deferred output projection of q-chunk qc: 8 elements."""
                els = []
                for qs in range(4):
                    state = {}

                    def el1(qs=qs, state=state, qc=qc):
                        py = aps.tile([P, 2, 512], F32, tag="ss",
                                      bufs=2, name=f"py{qc}_{qs}")
                        state["py"] = py
                        for c in range(4):
                            nc.tensor.matmul(
                                py[:, 0, :],
                                lhsT=ctx_tiles[qc][c][:, qs * 128:(qs + 1) * 128],
                                rhs=wo[:, c, 0:512],
                                start=(c == 0), stop=(c == 3))

                    def el2(qs=qs, state=state, qc=qc):
                        py = state["py"]
                        for c in range(4):
                            nc.tensor.matmul(
                                py[:, 1, :],
                                lhsT=ctx_tiles[qc][c][:, qs * 128:(qs + 1) * 128],
                                rhs=wo[:, c, 512:1024],
                                start=(c == 0), stop=(c == 3))
                        py_sb = pyp.tile([P, 2, 512], F16, tag="py",
                                         name=f"ysb{qc}_{qs}")
                        nc.scalar.activation(out=py_sb, in_=py, func=AF.Copy)
                        r0 = qc * 512 + qs * 128
                        nc.sync.dma_start(out=y_d[r0:r0 + 128, :],
                                          in_=py_sb.rearrange("p a b -> p (a b)"))

                    els += [el1, el2]
                return els

            for qc in range(NQC):
                if qc + 1 < NQC:
                    mq_tiles[qc + 1] = prefetch_mask(qc + 1)
                mq = mq_tiles[qc]
                fillers = []
                if qc + 1 < NQC:
                    xq = load_x_window(qc + 1, f"xq{qc + 1}")
                    fillers += qproj_fillers(qc + 1, xq)
                if qc > 0:
                    fillers += outproj_fillers(qc - 1)
                fi = 0
                q0 = qc * 512
                for pair in range(4):
                    pc = aps.tile([DH + 1, 2, 512], F32, tag="pc", bufs=2,
                                  name=f"pc{qc}_{pair}")
                    for kc in range(16):
                        u = pair * 16 + kc
                        ss = aps.tile([P, 2, 512], F32, tag="ss", bufs=2,
                                      name=f"ss{qc}_{u}")
                        for hp in range(2):
                            nc.tensor.matmul(
                                ss[:, hp, :],
                                lhsT=kT[hp * DH:(hp + 1) * DH, pair,
                                        kc * 128:(kc + 1) * 128],
                                rhs=qT[hp * DH:(hp + 1) * DH, pair,
                                       q0:q0 + 512],
                                start=True, stop=True)
                        if u % FILLER_EVERY == 2 and fi < len(fillers):
                            fillers[fi]()
                            fi += 1
                        nc.vector.tensor_tensor(ss[:, :, :], ss[:, :, :],
                                                mq[:, kc, :, :], OP.mult)
                        ex = exp_pool.tile([P, 2, 512], F16, tag="ex",
                                           name=f"ex{qc}_{u}")
                        nc.scalar.activation(out=ex, in_=ss,
                                             func=AF.Exp, scale=0.125)
                        for hp in range(2):
                            nc.tensor.matmul(
                                pc[:, hp, :],
                                lhsT=v[:, kc, 2 * pair + hp, :],
                                rhs=ex[:, hp, :],
                                start=(kc == 0), stop=(kc == 15))
                    # ---- normalize this head pair ----
                    rt = rtp.tile([1, 2, 512], F32, tag="rt",
                                  name=f"rt{qc}_{pair}")
                    nc.vector.reciprocal(rt, pc[DH:DH + 1, :, :])
                    rb = rbp.tile([DH, 2, 512], F32, tag="rb",
                                  name=f"rb{qc}_{pair}")
                    nc.gpsimd.partition_broadcast(rb, rt)
                    cpr = ctxp.tile([P, 512], F16, tag="cpr",
                                    name=f"cpr{qc}_{pair}")
                    nc.vector.tensor_tensor(cpr[0:DH, :], pc[0:DH, 0, :],
                                            rb[:, 0, :], OP.mult)
                    cxo = cxop.tile([DH, 512], F16, tag="cxo",
                                    name=f"cxo{qc}_{pair}")
                    nc.vector.tensor_tensor(cxo, pc[0:DH, 1, :],
                                            rb[:, 1, :], OP.mult)
                    sh = aps.tile([P, 2, 512], F32, tag="ss", bufs=2,
                                  name=f"sh{qc}_{pair}")
                    nc.tensor.matmul(sh[:, 0, :], lhsT=ident, rhs=cxo,
                                     start=True, stop=True)
                    nc.scalar.activation(out=cpr[DH:P, :], in_=sh[DH:P, 0, :],
                                         func=AF.Copy)
                    ctx_tiles[qc].append(cpr)
                # drain any unused fillers at end of this qc
                while fi < len(fillers):
                    fillers[fi]()
                    fi += 1
            # trailing output projection of the last q-chunk
            for el in outproj_fillers(NQC - 1):
                el()
    nc.compile()
    return nc


def _get_nc():
    if "nc" not in _CACHE:
        _CACHE["nc"] = _build()
    return _CACHE["nc"]


def kernel(input, mask, Wq, bq, Wk, bk, Wv, bv, Wo, bo):
    x = np.asarray(input, dtype=np.float32)
    m = np.asarray(mask, dtype=np.float32)
    Wq = np.asarray(Wq, dtype=np.float32)
    Wk = np.asarray(Wk, dtype=np.float32)
    Wv = np.asarray(Wv, dtype=np.float32)
    Wo = np.asarray(Wo, dtype=np.float32)
    bq = np.asarray(bq, dtype=np.float32)
    bk = np.asarray(bk, dtype=np.float32)
    bv = np.asarray(bv, dtype=np.float32)
    bo = np.asarray(bo, dtype=np.float32)
    B = x.shape[0]
    assert x.shape == (B, NSEQ, DMODEL) and B == 4

    nc = _get_nc()
    in_maps = []
    for b in range(B):
        xT = np.ascontiguousarray(x[b].T)
        mT = np.ascontiguousarray(m[b].T).astype(np.float16)
        for hg in range(2):
            sl = slice(hg * HD, (hg + 1) * HD)
            in_maps.append({
                "xT": xT.astype(np.float16),
                "maskT": mT,
                "wq": np.ascontiguousarray(Wq[:, sl]).astype(np.float16),
                "wk": np.ascontiguousarray(Wk[:, sl]).astype(np.float16),
                "wv": np.ascontiguousarray(Wv[:, sl]).astype(np.float16),
                "wo": np.ascontiguousarray(
                    Wo[sl].reshape(4, P, DMODEL).transpose(1, 0, 2)
                ).astype(np.float16),
                "ident": np.concatenate(
                    [np.zeros((DH, DH), np.float16),
                     np.eye(DH, dtype=np.float16)], axis=1),
                "bq2": np.ascontiguousarray(bq[sl].reshape(4, P).T),
                "bk2": np.ascontiguousarray(bk[sl].reshape(4, P).T),
                "bvr": np.ascontiguousarray(
                    np.broadcast_to(bv[sl], (P, HD))),
                "ones2": np.ones((P, 512), dtype=np.float32),
            })

    res = bass_utils.run_bass_kernel_spmd(nc, in_maps, core_ids=list(range(8)))
    global LAST_RESULTS
    LAST_RESULTS = res

    out = np.empty((B, NSEQ, DMODEL), dtype=np.float32)
    for b in range(B):
        out[b] = (res.results[2 * b]["y"].astype(np.float32)
                  + res.results[2 * b + 1]["y"].astype(np.float32) + bo)
    return out


# revision 26
# speedup vs baseline: 1.2805x; 1.2805x over previous
"""Multi-head attention (B=4, N=2048, D=1024, H=16) on 8 Trainium2 NeuronCores.

Sharding: core c = 2*b + hg handles batch b and head-group hg (8 of 16 heads).
Host pre-transposes x and mask per batch into DMA-friendly contiguous layouts,
slices Wq/Wk/Wv columns and Wo rows per head group, and sums the two partial
outputs per batch (+ bo) at the end.

v3 design notes:
  - The PE only reaches 2.4 GHz after ~3us of uninterrupted work, so the
    attention stream is software-pipelined: ctx matmuls are emitted LAG units
    behind their scores/mult/exp producers so the in-order PE queue never
    blocks on the DVE->ACT chain; per-pair softmax normalization is deferred
    into the next pair's unit stream.
  - q-projection for q-chunks 1..3 and the output projection are deferred and
    interleaved into the (elementwise-bound) attention stream as PE filler.
  - Mask is fp16 and host-packed [qc, p, kc, 512] so each DMA is one
    16KB-per-partition contiguous descriptor set; x and weights likewise.
  - Softmax: ctx matmul lhsT is v augmented with a ones column -> psum row 64
    accumulates the denominator. DVE reciprocal -> tiny PE ones-matmul
    broadcasts it across 64 partitions (GpSimd partition_broadcast has ~10us
    dispatch latency; PE does it in 0.2us) -> DVE multiplies, PE ident-shift
    packs the second head into rows 64..127.
  - PSUM: scores ring 2x[128,2,512] (4 banks) + ctx ring 2x[65,2,512] (4).
    Normalize/out-proj/q-proj psum shares the scores ring.
"""
import os
from contextlib import ExitStack

import numpy as np

from concourse import bacc, mybir, tile
from concourse import bass_utils

P = 128
NSEQ = 2048          # sequence length
DMODEL = 1024        # model dim
HD = 512             # per-core head dim total (8 heads x 64)
NH = 8               # heads per core
DH = 64              # head depth
DC = DMODEL // P     # 8 d_model chunks
NQC = 4              # q chunks of 512
F32 = mybir.dt.float32
F32R = mybir.dt.float32r
F16 = mybir.dt.float16
AF = mybir.ActivationFunctionType
OP = mybir.AluOpType

N_WARM = 12          # PE clock warm-up matmuls
FILLER_EVERY = 4     # emit one filler element every this many units
LAG = 2              # units between scores emission and ctx emission

_CACHE: dict = {}
LAST_RESULTS = None


def _patch_act_tables():
    """Force every activation onto the one table set containing
    exp+ln+copy+identity, so the kernel performs a single ACT_TABLE_LOAD
    instead of thrashing between per-function sets (1.3us each)."""
    import functools
    from concourse import bacc as _bacc
    from concourse import hw_specs as _hw
    if getattr(_bacc, "_act_tables_patched", False):
        return
    orig = _hw.get_activation_tables

    @functools.cache
    def patched(arch):
        tabs = dict(orig(arch))
        full = "natural_log_exp_and_others"
        keep = tabs[full]
        strip = {f for f in keep}
        out = {}
        for name, funcs in tabs.items():
            out[name] = funcs if name == full else (funcs - strip)
        return out

    _bacc.get_activation_tables = patched
    _bacc._act_tables_patched = True


def _build():
    _patch_act_tables()
    nc = bacc.Bacc("TRN2", target_bir_lowering=False, debug=False,
                   enable_asserts=False, num_devices=8)

    x_d = nc.dram_tensor("xp", [NQC, P, DC, 512], F16, kind="ExternalInput").ap()
    mask_d = nc.dram_tensor("maskp", [NQC, P, 16, 512], F16,
                            kind="ExternalInput").ap()
    wq_d = nc.dram_tensor("wq", [P, DC, 512], F16, kind="ExternalInput").ap()
    wk_d = nc.dram_tensor("wk", [P, DC, 512], F16, kind="ExternalInput").ap()
    wv_d = nc.dram_tensor("wv", [P, DC, 512], F16, kind="ExternalInput").ap()
    wo_d = nc.dram_tensor("wo", [P, 4, DMODEL], F16, kind="ExternalInput").ap()
    id_d = nc.dram_tensor("ident", [DH, P], F16, kind="ExternalInput").ap()
    bq_d = nc.dram_tensor("bq2", [P, 4], F32, kind="ExternalInput").ap()
    bk_d = nc.dram_tensor("bk2", [P, 4], F32, kind="ExternalInput").ap()
    bvr_d = nc.dram_tensor("bvr", [P, HD], F32, kind="ExternalInput").ap()
    ones_d = nc.dram_tensor("ones2", [P, 512], F32R, kind="ExternalInput").ap()
    y_d = nc.dram_tensor("y", [NSEQ, DMODEL], F16, kind="ExternalOutput").ap()

    with tile.TileContext(nc) as tc, ExitStack() as ctx:
        persist = ctx.enter_context(tc.tile_pool(name="persist", bufs=1))
        qT = persist.tile([P, 4, NSEQ], F16)     # [hd%128, hd-chunk, seq]
        kT = persist.tile([P, 4, NSEQ], F16)
        v = persist.tile([P, 16, NH, DH + 1], F16)  # [seq%128, seq-chunk, h, d|1]
        wq = persist.tile([P, DC, HD], F16)
        wk = persist.tile([P, DC, HD], F16)
        wv = persist.tile([P, DC, HD], F16)
        wo = persist.tile([P, 4, DMODEL], F16)
        ident = persist.tile([DH, P], F16)
        ones = persist.tile([P, 512], F32R)
        bqs = persist.tile([P, 4], F32)
        bks = persist.tile([P, 4], F32)
        bvr = persist.tile([P, HD], F32)

        mqp = ctx.enter_context(tc.tile_pool(name="mq", bufs=2))
        exp_pool = ctx.enter_context(tc.tile_pool(name="ex", bufs=4))
        ctxp = ctx.enter_context(tc.tile_pool(name="cpr", bufs=8))
        rtp = ctx.enter_context(tc.tile_pool(name="rt", bufs=2))
        rbsbp = ctx.enter_context(tc.tile_pool(name="rbsb", bufs=2))
        cxop = ctx.enter_context(tc.tile_pool(name="cxo", bufs=2))
        pyp = ctx.enter_context(tc.tile_pool(name="py_sb", bufs=2))
        xwp = ctx.enter_context(tc.tile_pool(name="xw", bufs=2))

        def load_x_window(n, name):
            xw = xwp.tile([P, DC, 512], F16, tag="xw", name=name)
            nc.sync.dma_start(out=xw, in_=x_d[n])
            return xw

        # ---- initial DMAs (queue order matters: earliest-needed first) ----
        nc.sync.dma_start(out=ones, in_=ones_d)
        xw0 = load_x_window(0, "xn0")
        nc.sync.dma_start(out=wk, in_=wk_d)
        nc.sync.dma_start(out=wv, in_=wv_d)
        nc.sync.dma_start(out=wq, in_=wq_d)
        nc.sync.dma_start(out=bqs, in_=bq_d)
        nc.sync.dma_start(out=bks, in_=bk_d)
        nc.sync.dma_start(out=bvr, in_=bvr_d)
        nc.sync.dma_start(out=ident, in_=id_d)
        nc.sync.dma_start(out=wo, in_=wo_d)

        nc.vector.tensor_copy(
            out=v[:, :, :, DH],
            in_=ones[:, 0:P].rearrange("p (a b) -> p a b", b=NH))

        def prefetch_mask(qc):
            mq = mqp.tile([P, 2, 16, 512], F16, tag="mq", name=f"mq{qc}")
            for hp in range(2):
                nc.sync.dma_start(out=mq[:, hp, :, :], in_=mask_d[qc])
            return mq

        # ---------------- phase P: projections ----------------
        with tc.tile_pool(name="pp", bufs=6, space="PSUM") as pp:
            wt = pp.tile([P, 512], F32, tag="pp", name="warm")
            for i in range(N_WARM):
                nc.tensor.matmul(wt, lhsT=ones[:, 0:P], rhs=ones,
                                 start=(i == 0), stop=(i == N_WARM - 1))
            mq_tiles = {0: prefetch_mask(0)}
            for n in range(NQC):
                x_n = xw0 if n == 0 else load_x_window(n, f"xn{n}")
                for m in range(4):
                    ps = pp.tile([P, 512], F32, tag="pp", name="psk")
                    for dc in range(DC):
                        nc.tensor.matmul(ps,
                                         lhsT=wk[:, dc, m * 128:(m + 1) * 128],
                                         rhs=x_n[:, dc, :],
                                         start=(dc == 0), stop=(dc == DC - 1))
                    nc.scalar.activation(
                        out=kT[:, m, n * 512:(n + 1) * 512], in_=ps,
                        func=AF.Identity, bias=bks[:, m:m + 1], scale=1.0)
                for s4 in range(4):
                    psv = pp.tile([P, 512], F32, tag="pp", name="psv")
                    for dc in range(DC):
                        nc.tensor.matmul(psv,
                                         lhsT=x_n[:, dc, s4 * 128:(s4 + 1) * 128],
                                         rhs=wv[:, dc, :],
                                         start=(dc == 0), stop=(dc == DC - 1))
                    s = n * 4 + s4
                    nc.vector.tensor_tensor(
                        v[:, s, :, 0:DH],
                        psv.rearrange("p (h d) -> p h d", d=DH),
                        bvr.rearrange("p (h d) -> p h d", d=DH),
                        OP.add)
                if n == 0:
                    for m in range(4):
                        psq = pp.tile([P, 512], F32, tag="pp", name="psq0")
                        for dc in range(DC):
                            nc.tensor.matmul(psq,
                                             lhsT=wq[:, dc, m * 128:(m + 1) * 128],
                                             rhs=x_n[:, dc, :],
                                             start=(dc == 0), stop=(dc == DC - 1))
                        nc.scalar.activation(
                            out=qT[:, m, 0:512], in_=psq,
                            func=AF.Identity, bias=bqs[:, m:m + 1], scale=1.0)

        # ---------------- attention + output projection ----------------
        ctx_tiles = {qc: [] for qc in range(NQC)}
        with tc.tile_pool(name="aps", bufs=1, space="PSUM") as aps:

            def qproj_fillers(qc, xq):
                els = []
                for m in range(4):
                    state = {}

                    def el1(m=m, state=state, xq=xq):
                        psq = aps.tile([P, 2, 512], F32, tag="ss",
                                       bufs=2, name=f"psq{qc}_{m}")
                        state["psq"] = psq
                        for dc in range(4):
                            nc.tensor.matmul(
                                psq[:, 0, :],
                                lhsT=wq[:, dc, m * 128:(m + 1) * 128],
                                rhs=xq[:, dc, :],
                                start=(dc == 0), stop=False)

                    def el2(m=m, state=state, qc=qc, xq=xq):
                        psq = state["psq"]
                        for dc in range(4, DC):
                            nc.tensor.matmul(
                                psq[:, 0, :],
                                lhsT=wq[:, dc, m * 128:(m + 1) * 128],
                                rhs=xq[:, dc, :],
                                start=False, stop=(dc == DC - 1))
                        nc.scalar.activation(
                            out=qT[:, m, qc * 512:(qc + 1) * 512],
                            in_=psq[:, 0, :],
                            func=AF.Identity, bias=bqs[:, m:m + 1], scale=1.0)

                    els += [el1, el2]
                return els

            def outproj_fillers(qc):
                els = []
                for qs in range(4):
                    state = {}

                    def el1(qs=qs, state=state, qc=qc):
                        py = aps.tile([P, 2, 512], F32, tag="ss",
                                      bufs=2, name=f"py{qc}_{qs}")
                        state["py"] = py
                        for c in range(4):
                            nc.tensor.matmul(
                                py[:, 0, :],
                                lhsT=ctx_tiles[qc][c][:, qs * 128:(qs + 1) * 128],
                                rhs=wo[:, c, 0:512],
                                start=(c == 0), stop=(c == 3))

                    def el2(qs=qs, state=state, qc=qc):
                        py = state["py"]
                        for c in range(4):
                            nc.tensor.matmul(
                                py[:, 1, :],
                                lhsT=ctx_tiles[qc][c][:, qs * 128:(qs + 1) * 128],
                                rhs=wo[:, c, 512:1024],
                                start=(c == 0), stop=(c == 3))
                        py_sb = pyp.tile([P, 2, 512], F16, tag="py",
                                         name=f"ysb{qc}_{qs}")
                        nc.scalar.activation(out=py_sb, in_=py, func=AF.Copy)
                        r0 = qc * 512 + qs * 128
                        nc.sync.dma_start(out=y_d[r0:r0 + 128, :],
                                          in_=py_sb.rearrange("p a b -> p (a b)"))

                    els += [el1, el2]
                return els

            # ---- software-pipelined unit stream ----
            NU = NQC * 4 * 16  # 256 units
            ex_tiles = {}      # u -> exp tile
            pc_tiles = {}      # (qc, pair) -> ctx psum accumulator
            deferred = {}      # step -> list of closures (normalize pieces)
            qc_state = {}

            def unit(u):
                return u // 64, (u // 16) % 4, u % 16  # qc, pair, kc

            def emit_ctx(u, step):
                qc, pair, kc = unit(u)
                key = (qc, pair)
                if key not in pc_tiles:
                    pc_tiles[key] = aps.tile([DH + 1, 2, 512], F32, tag="pc",
                                             bufs=2, name=f"pc{qc}_{pair}")
                pc = pc_tiles[key]
                ex = ex_tiles.pop(u)
                for hp in range(2):
                    nc.tensor.matmul(
                        pc[:, hp, :],
                        lhsT=v[:, kc, 2 * pair + hp, :],
                        rhs=ex[:, hp, :],
                        start=(kc == 0), stop=(kc == 15))
                if kc == 15:
                    schedule_normalize(step, qc, pair, pc)

            def schedule_normalize(step, qc, pair, pc):
                """Emit recip now; broadcast+mults and shift at later steps."""
                rt = rtp.tile([1, 2, 512], F32R, tag="rt",
                              name=f"rt{qc}_{pair}")
                with nc.allow_low_precision(reason="f32r is 32-bit storage"):
                    nc.vector.reciprocal(rt, pc[DH:DH + 1, :, :])
                cpr = ctxp.tile([P, 512], F16, tag="cpr",
                                name=f"cpr{qc}_{pair}")
                ctx_tiles[qc].append(cpr)

                def piece2():
                    rb = aps.tile([P, 2, 512], F32, tag="ss", bufs=2,
                                  name=f"rb{qc}_{pair}")
                    for hp in range(2):
                        nc.tensor.matmul(rb[0:DH, hp, :],
                                         lhsT=ones[0:1, 0:DH],
                                         rhs=rt[0:1, hp, :],
                                         start=True, stop=True)
                    rb_sb = rbsbp.tile([DH, 2, 512], F16, tag="rb",
                                       name=f"rbsb{qc}_{pair}")
                    state[0] = rb_sb
                    nc.scalar.activation(out=rb_sb, in_=rb[0:DH, :, :],
                                         func=AF.Copy)

                def piece3():
                    rb_sb = state[0]
                    nc.vector.tensor_tensor(cpr[0:DH, :], pc[0:DH, 0, :],
                                            rb_sb[:, 0, :], OP.mult)
                    cxo = cxop.tile([DH, 512], F16, tag="cxo",
                                    name=f"cxo{qc}_{pair}")
                    state[1] = cxo
                    nc.vector.tensor_tensor(cxo, pc[0:DH, 1, :],
                                            rb_sb[:, 1, :], OP.mult)

                def piece4():
                    cxo = state[1]
                    sh = aps.tile([P, 2, 512], F32, tag="ss", bufs=2,
                                  name=f"sh{qc}_{pair}")
                    nc.tensor.matmul(sh[:, 0, :], lhsT=ident, rhs=cxo,
                                     start=True, stop=True)
                    nc.scalar.activation(out=cpr[DH:P, :],
                                         in_=sh[DH:P, 0, :], func=AF.Copy)

                state = [None, None]
                deferred.setdefault(step + 1, []).append(piece2)
                deferred.setdefault(step + 2, []).append(piece3)
                deferred.setdefault(step + 3, []).append(piece4)

            for u in range(NU):
                qc, pair, kc = unit(u)
                if pair == 0 and kc == 0:
                    # qc start: prefetch next mask, build filler queue
                    if qc + 1 < NQC:
                        mq_tiles[qc + 1] = prefetch_mask(qc + 1)
                    fillers = []
                    if qc + 1 < NQC:
                        xq = load_x_window(qc + 1, f"xq{qc + 1}")
                        fillers += qproj_fillers(qc + 1, xq)
                    if qc > 0:
                        fillers += outproj_fillers(qc - 1)
                    qc_state[qc] = {"fillers": fillers, "fi": 0}
                st = qc_state[qc]
                mq = mq_tiles[qc]

                ss = aps.tile([P, 2, 512], F32, tag="ss", bufs=2,
                              name=f"ss{qc}_{pair}_{kc}")
                for hp in range(2):
                    nc.tensor.matmul(
                        ss[:, hp, :],
                        lhsT=kT[hp * DH:(hp + 1) * DH, pair,
                                kc * 128:(kc + 1) * 128],
                        rhs=qT[hp * DH:(hp + 1) * DH, pair,
                               qc * 512:(qc + 1) * 512],
                        start=True, stop=True)
                nc.vector.tensor_tensor(ss[:, :, :], ss[:, :, :],
                                        mq[:, :, kc, :], OP.mult)
                ex = exp_pool.tile([P, 2, 512], F16, tag="ex",
                                   name=f"ex{qc}_{pair}_{kc}")
                nc.scalar.activation(out=ex, in_=ss, func=AF.Exp, scale=0.125)
                ex_tiles[u] = ex
                if u >= LAG:
                    emit_ctx(u - LAG, u)
                for fn in deferred.pop(u, ()):
                    fn()
                # Fillers only after the first 8 units of a qc: the previous
                # qc's last cpr tile is written by deferred normalize pieces
                # in the first ~4 steps, and reads emitted before those
                # writes would not be dependency-tracked.
                if (u % 64) >= 8 and u % FILLER_EVERY == 1 \
                        and st["fi"] < len(st["fillers"]):
                    st["fillers"][st["fi"]]()
                    st["fi"] += 1
                # drain leftover fillers at the very end of each qc
                if kc == 15 and pair == 3:
                    while st["fi"] < len(st["fillers"]):
                        st["fillers"][st["fi"]]()
                        st["fi"] += 1

            for i, u in enumerate(range(NU - LAG, NU)):
                step = NU + i
                emit_ctx(u, step)
                for fn in deferred.pop(step, ()):
                    fn()
            for step in sorted(deferred):
                for fn in deferred.pop(step):
                    fn()
            for el in outproj_fillers(NQC - 1):
                el()
    nc.compile()
    return nc


def _get_nc():
    if "nc" not in _CACHE:
        _CACHE["nc"] = _build()
    return _CACHE["nc"]


def kernel(input, mask, Wq, bq, Wk, bk, Wv, bv, Wo, bo):
    x = np.asarray(input, dtype=np.float32)
    m = np.asarray(mask, dtype=np.float32)
    Wq = np.asarray(Wq, dtype=np.float32)
    Wk = np.asarray(Wk, dtype=np.float32)
    Wv = np.asarray(Wv, dtype=np.float32)
    Wo = np.asarray(Wo, dtype=np.float32)
    bq = np.asarray(bq, dtype=np.float32)
    bk = np.asarray(bk, dtype=np.float32)
    bv = np.asarray(bv, dtype=np.float32)
    bo = np.asarray(bo, dtype=np.float32)
    B = x.shape[0]
    assert x.shape == (B, NSEQ, DMODEL) and B == 4

    nc = _get_nc()

    def pack_w(W):  # [1024, 512] -> [p, dc, m]
        return np.ascontiguousarray(
            W.reshape(DC, P, HD).transpose(1, 0, 2)).astype(np.float16)

    in_maps = []
    for b in range(B):
        xT = x[b].T.astype(np.float16)            # [1024, 2048]
        x_pre = np.ascontiguousarray(
            xT.reshape(DC, P, NQC, 512).transpose(2, 1, 0, 3))
        mT = m[b].T.astype(np.float16)            # [k, q]
        m_pre = np.ascontiguousarray(
            mT.reshape(16, P, NQC, 512).transpose(2, 1, 0, 3))
        for hg in range(2):
            sl = slice(hg * HD, (hg + 1) * HD)
            in_maps.append({
                "xp": x_pre,
                "maskp": m_pre,
                "wq": pack_w(Wq[:, sl]),
                "wk": pack_w(Wk[:, sl]),
                "wv": pack_w(Wv[:, sl]),
                "wo": np.ascontiguousarray(
                    Wo[sl].reshape(4, P, DMODEL).transpose(1, 0, 2)
                ).astype(np.float16),
                "ident": np.concatenate(
                    [np.zeros((DH, DH), np.float16),
                     np.eye(DH, dtype=np.float16)], axis=1),
                "bq2": np.ascontiguousarray(bq[sl].reshape(4, P).T),
                "bk2": np.ascontiguousarray(bk[sl].reshape(4, P).T),
                "bvr": np.ascontiguousarray(
                    np.broadcast_to(bv[sl], (P, HD))),
                "ones2": np.ones((P, 512), dtype=np.float32),
            })

    res = bass_utils.run_bass_kernel_spmd(nc, in_maps, core_ids=list(range(8)))
    global LAST_RESULTS
    LAST_RESULTS = res

    out = np.empty((B, NSEQ, DMODEL), dtype=np.float32)
    for b in range(B):
        out[b] = (res.results[2 * b]["y"].astype(np.float32)
                  + res.results[2 * b + 1]["y"].astype(np.float32) + bo)
    return out


# revision 37
# speedup vs baseline: 1.7206x; 1.3437x over previous
"""Multi-head attention (B=4, N=2048, D=1024, H=16) on 8 Trainium2 NeuronCores.

Sharding: core c = 2*b + hg handles batch b and head-group hg (8 of 16 heads).
Host pre-transposes x and mask per batch into DMA-friendly contiguous layouts,
slices Wq/Wk/Wv columns and Wo rows per head group, and sums the two partial
outputs per batch (+ bo) at the end.

v3 design notes:
  - The PE only reaches 2.4 GHz after ~3us of uninterrupted work, so the
    attention stream is software-pipelined: ctx matmuls are emitted LAG units
    behind their scores/mult/exp producers so the in-order PE queue never
    blocks on the DVE->ACT chain; per-pair softmax normalization is deferred
    into the next pair's unit stream.
  - q-projection for q-chunks 1..3 and the output projection are deferred and
    interleaved into the (elementwise-bound) attention stream as PE filler.
  - Mask is fp16 and host-packed [qc, p, kc, 512] so each DMA is one
    16KB-per-partition contiguous descriptor set; x and weights likewise.
  - Softmax: ctx matmul lhsT is v augmented with a ones column -> psum row 64
    accumulates the denominator. DVE reciprocal -> tiny PE ones-matmul
    broadcasts it across 64 partitions (GpSimd partition_broadcast has ~10us
    dispatch latency; PE does it in 0.2us) -> DVE multiplies, PE ident-shift
    packs the second head into rows 64..127.
  - PSUM: scores ring 2x[128,2,512] (4 banks) + ctx ring 2x[65,2,512] (4).
    Normalize/out-proj/q-proj psum shares the scores ring.
"""
import os
from contextlib import ExitStack

import numpy as np

from concourse import bacc, mybir, tile
from concourse import bass_utils

P = 128
NSEQ = 2048          # sequence length
DMODEL = 1024        # model dim
HD = 512             # per-core head dim total (8 heads x 64)
NH = 8               # heads per core
DH = 64              # head depth
DC = DMODEL // P     # 8 d_model chunks
NQC = 4              # q chunks of 512
F32 = mybir.dt.float32
F32R = mybir.dt.float32r
F16 = mybir.dt.float16
AF = mybir.ActivationFunctionType
OP = mybir.AluOpType

N_WARM = 12          # PE clock warm-up matmuls
FILLER_EVERY = 4     # emit one filler element every this many units
LAG = 2              # units between scores emission and ctx emission

_CACHE: dict = {}
LAST_RESULTS = None


def _patch_act_tables():
    """Force every activation onto the one table set containing
    exp+ln+copy+identity, so the kernel performs a single ACT_TABLE_LOAD
    instead of thrashing between per-function sets (1.3us each)."""
    import functools
    from concourse import bacc as _bacc
    from concourse import hw_specs as _hw
    if getattr(_bacc, "_act_tables_patched", False):
        return
    orig = _hw.get_activation_tables

    @functools.cache
    def patched(arch):
        tabs = dict(orig(arch))
        full = "natural_log_exp_and_others"
        keep = tabs[full]
        strip = {f for f in keep}
        out = {}
        for name, funcs in tabs.items():
            out[name] = funcs if name == full else (funcs - strip)
        return out

    _bacc.get_activation_tables = patched
    _bacc._act_tables_patched = True


def _build():
    _patch_act_tables()
    nc = bacc.Bacc("TRN2", target_bir_lowering=False, debug=False,
                   enable_asserts=False, num_devices=8)

    x_d = nc.dram_tensor("xp", [NQC, P, DC, 512], F16, kind="ExternalInput").ap()
    mask_d = nc.dram_tensor("maskp", [NQC, P, 16, 512], F16,
                            kind="ExternalInput").ap()
    wq_d = nc.dram_tensor("wq", [P, DC, 512], F16, kind="ExternalInput").ap()
    wk_d = nc.dram_tensor("wk", [P, DC, 512], F16, kind="ExternalInput").ap()
    wv_d = nc.dram_tensor("wv", [P, DC, 512], F16, kind="ExternalInput").ap()
    wo_d = nc.dram_tensor("wo", [P, 4, DMODEL], F16, kind="ExternalInput").ap()
    id_d = nc.dram_tensor("ident", [DH, P], F16, kind="ExternalInput").ap()
    bq_d = nc.dram_tensor("bq2", [P, 4], F32, kind="ExternalInput").ap()
    bk_d = nc.dram_tensor("bk2", [P, 4], F32, kind="ExternalInput").ap()
    bvr_d = nc.dram_tensor("bvr", [P, HD], F32, kind="ExternalInput").ap()
    ones_d = nc.dram_tensor("ones2", [P, 512], F32R, kind="ExternalInput").ap()
    y_d = nc.dram_tensor("y", [NSEQ, DMODEL], F16, kind="ExternalOutput").ap()

    with tile.TileContext(nc) as tc, ExitStack() as ctx:
        persist = ctx.enter_context(tc.tile_pool(name="persist", bufs=1))
        qT = persist.tile([P, 4, NSEQ], F16)     # [hd%128, hd-chunk, seq]
        kT = persist.tile([P, 4, NSEQ], F16)
        v = persist.tile([P, 16, NH, DH + 1], F16)  # [seq%128, seq-chunk, h, d|1]
        wq = persist.tile([P, DC, HD], F16)
        wk = persist.tile([P, DC, HD], F16)
        wv = persist.tile([P, DC, HD], F16)
        wo = persist.tile([P, 4, DMODEL], F16)
        ident = persist.tile([DH, P], F16)
        ones = persist.tile([P, 512], F32R)
        bqs = persist.tile([P, 4], F32)
        bks = persist.tile([P, 4], F32)
        bvr = persist.tile([P, HD], F32)

        mqp = ctx.enter_context(tc.tile_pool(name="mq", bufs=2))
        exp_pool = ctx.enter_context(tc.tile_pool(name="ex", bufs=3))
        smp = ctx.enter_context(tc.tile_pool(name="sm", bufs=3))
        ctxp = ctx.enter_context(tc.tile_pool(name="cpr", bufs=8))
        rtp = ctx.enter_context(tc.tile_pool(name="rt", bufs=1))
        rbsbp = ctx.enter_context(tc.tile_pool(name="rbsb", bufs=2))
        cxop = ctx.enter_context(tc.tile_pool(name="cxo", bufs=1))
        pyp = ctx.enter_context(tc.tile_pool(name="py_sb", bufs=2))
        xwp = ctx.enter_context(tc.tile_pool(name="xw", bufs=2))

        def load_x_window(n, name):
            xw = xwp.tile([P, DC, 512], F16, tag="xw", name=name)
            nc.sync.dma_start(out=xw, in_=x_d[n])
            return xw

        # ---- initial DMAs (queue order matters: earliest-needed first) ----
        nc.sync.dma_start(out=ones, in_=ones_d)
        xw0 = load_x_window(0, "xn0")
        nc.sync.dma_start(out=wk, in_=wk_d)
        nc.sync.dma_start(out=wv, in_=wv_d)
        nc.sync.dma_start(out=wq, in_=wq_d)
        nc.sync.dma_start(out=bqs, in_=bq_d)
        nc.sync.dma_start(out=bks, in_=bk_d)
        nc.sync.dma_start(out=bvr, in_=bvr_d)
        nc.sync.dma_start(out=ident, in_=id_d)
        nc.sync.dma_start(out=wo, in_=wo_d)

        nc.vector.tensor_copy(
            out=v[:, :, :, DH],
            in_=ones[:, 0:P].rearrange("p (a b) -> p a b", b=NH))

        def prefetch_mask(qc):
            mq = mqp.tile([P, 2, 16, 512], F16, tag="mq", name=f"mq{qc}")
            for hp in range(2):
                nc.sync.dma_start(out=mq[:, hp, :, :], in_=mask_d[qc])
            return mq

        # ---------------- phase P: projections ----------------
        with tc.tile_pool(name="pp", bufs=6, space="PSUM") as pp:
            wt = pp.tile([P, 512], F32, tag="pp", name="warm")
            for i in range(N_WARM):
                nc.tensor.matmul(wt, lhsT=ones[:, 0:P], rhs=ones,
                                 start=(i == 0), stop=(i == N_WARM - 1))
            mq_tiles = {0: prefetch_mask(0)}
            for n in range(NQC):
                x_n = xw0 if n == 0 else load_x_window(n, f"xn{n}")
                for m in range(4):
                    ps = pp.tile([P, 512], F32, tag="pp", name="psk")
                    for dc in range(DC):
                        nc.tensor.matmul(ps,
                                         lhsT=wk[:, dc, m * 128:(m + 1) * 128],
                                         rhs=x_n[:, dc, :],
                                         start=(dc == 0), stop=(dc == DC - 1))
                    nc.scalar.activation(
                        out=kT[:, m, n * 512:(n + 1) * 512], in_=ps,
                        func=AF.Identity, bias=bks[:, m:m + 1], scale=1.0)
                for s4 in range(4):
                    psv = pp.tile([P, 512], F32, tag="pp", name="psv")
                    for dc in range(DC):
                        nc.tensor.matmul(psv,
                                         lhsT=x_n[:, dc, s4 * 128:(s4 + 1) * 128],
                                         rhs=wv[:, dc, :],
                                         start=(dc == 0), stop=(dc == DC - 1))
                    s = n * 4 + s4
                    nc.vector.tensor_tensor(
                        v[:, s, :, 0:DH],
                        psv.rearrange("p (h d) -> p h d", d=DH),
                        bvr.rearrange("p (h d) -> p h d", d=DH),
                        OP.add)
                if n == 0:
                    for m in range(4):
                        psq = pp.tile([P, 512], F32, tag="pp", name="psq0")
                        for dc in range(DC):
                            nc.tensor.matmul(psq,
                                             lhsT=wq[:, dc, m * 128:(m + 1) * 128],
                                             rhs=x_n[:, dc, :],
                                             start=(dc == 0), stop=(dc == DC - 1))
                        nc.scalar.activation(
                            out=qT[:, m, 0:512], in_=psq,
                            func=AF.Identity, bias=bqs[:, m:m + 1], scale=1.0)

        # ---------------- attention + output projection ----------------
        ctx_tiles = {qc: [] for qc in range(NQC)}
        with tc.tile_pool(name="aps", bufs=1, space="PSUM") as aps:

            def qproj_fillers(qc, xq):
                els = []
                for m in range(4):
                    state = {}

                    def el1(m=m, state=state, xq=xq):
                        psq = aps.tile([P, 2, 512], F32, tag="ss",
                                       bufs=2, name=f"psq{qc}_{m}")
                        state["psq"] = psq
                        for dc in range(4):
                            nc.tensor.matmul(
                                psq[:, 0, :],
                                lhsT=wq[:, dc, m * 128:(m + 1) * 128],
                                rhs=xq[:, dc, :],
                                start=(dc == 0), stop=False)

                    def el2(m=m, state=state, qc=qc, xq=xq):
                        psq = state["psq"]
                        for dc in range(4, DC):
                            nc.tensor.matmul(
                                psq[:, 0, :],
                                lhsT=wq[:, dc, m * 128:(m + 1) * 128],
                                rhs=xq[:, dc, :],
                                start=False, stop=(dc == DC - 1))
                        nc.scalar.activation(
                            out=qT[:, m, qc * 512:(qc + 1) * 512],
                            in_=psq[:, 0, :],
                            func=AF.Identity, bias=bqs[:, m:m + 1], scale=1.0)

                    els += [el1, el2]
                return els

            def outproj_fillers(qc):
                els = []
                for qs in range(4):
                    state = {}

                    def el1(qs=qs, state=state, qc=qc):
                        py = aps.tile([P, 2, 512], F32, tag="ss",
                                      bufs=2, name=f"py{qc}_{qs}")
                        state["py"] = py
                        for c in range(4):
                            nc.tensor.matmul(
                                py[:, 0, :],
                                lhsT=ctx_tiles[qc][c][:, qs * 128:(qs + 1) * 128],
                                rhs=wo[:, c, 0:512],
                                start=(c == 0), stop=(c == 3))

                    def el2(qs=qs, state=state, qc=qc):
                        py = state["py"]
                        for c in range(4):
                            nc.tensor.matmul(
                                py[:, 1, :],
                                lhsT=ctx_tiles[qc][c][:, qs * 128:(qs + 1) * 128],
                                rhs=wo[:, c, 512:1024],
                                start=(c == 0), stop=(c == 3))
                        py_sb = pyp.tile([P, 2, 512], F16, tag="py",
                                         name=f"ysb{qc}_{qs}")
                        nc.scalar.activation(out=py_sb, in_=py, func=AF.Copy)
                        r0 = qc * 512 + qs * 128
                        nc.sync.dma_start(out=y_d[r0:r0 + 128, :],
                                          in_=py_sb.rearrange("p a b -> p (a b)"))

                    els += [el1, el2]
                return els

            # ---- software-pipelined unit stream ----
            NU = NQC * 4 * 16  # 256 units
            ex_tiles = {}      # u -> exp tile
            pc_tiles = {}      # (qc, pair) -> ctx psum accumulator
            deferred = {}      # step -> list of closures (normalize pieces)
            qc_state = {}

            def unit(u):
                return u // 64, (u // 16) % 4, u % 16  # qc, pair, kc

            def emit_ctx(u, step):
                qc, pair, kc = unit(u)
                key = (qc, pair)
                if key not in pc_tiles:
                    pc_tiles[key] = aps.tile([DH + 1, 2, 512], F32, tag="pc",
                                             bufs=2, name=f"pc{qc}_{pair}")
                pc = pc_tiles[key]
                ex = ex_tiles.pop(u)
                for hp in range(2):
                    nc.tensor.matmul(
                        pc[:, hp, :],
                        lhsT=v[:, kc, 2 * pair + hp, :],
                        rhs=ex[:, hp, :],
                        start=(kc == 0), stop=(kc == 15))
                if kc == 15:
                    schedule_normalize(step, qc, pair, pc)

            def schedule_normalize(step, qc, pair, pc):
                """Emit recip now; broadcast+mults and shift at later steps."""
                dd = rtp.tile([1, 2, 512], F32R, tag="rt",
                              name=f"dd{qc}_{pair}")
                nc.scalar.activation(out=dd, in_=pc[DH:DH + 1, :, :],
                                     func=AF.Copy)
                cpr = ctxp.tile([P, 512], F16, tag="cpr",
                                name=f"cpr{qc}_{pair}")
                ctx_tiles[qc].append(cpr)

                def piece2():
                    rb = aps.tile([P, 2, 512], F32, tag="ss", bufs=2,
                                  name=f"rb{qc}_{pair}")
                    for hp in range(2):
                        nc.tensor.matmul(rb[0:DH, hp, :],
                                         lhsT=ones[0:1, 0:DH],
                                         rhs=dd[0:1, hp, :],
                                         start=True, stop=True)
                    rb_sb = rbsbp.tile([DH, 2, 512], F32, tag="rb",
                                       name=f"rbsb{qc}_{pair}")
                    state[0] = rb_sb
                    nc.vector.reciprocal_approx_fast(rb_sb, rb[0:DH, :, :])

                def piece3():
                    rb_sb = state[0]
                    nc.vector.tensor_tensor(cpr[0:DH, :], pc[0:DH, 0, :],
                                            rb_sb[:, 0, :], OP.mult)
                    cxo = cxop.tile([DH, 512], F16, tag="cxo",
                                    name=f"cxo{qc}_{pair}")
                    state[1] = cxo
                    nc.vector.tensor_tensor(cxo, pc[0:DH, 1, :],
                                            rb_sb[:, 1, :], OP.mult)

                def piece4():
                    cxo = state[1]
                    sh = aps.tile([P, 2, 512], F32, tag="ss", bufs=2,
                                  name=f"sh{qc}_{pair}")
                    nc.tensor.matmul(sh[:, 0, :], lhsT=ident, rhs=cxo,
                                     start=True, stop=True)
                    nc.scalar.activation(out=cpr[DH:P, :],
                                         in_=sh[DH:P, 0, :], func=AF.Copy)

                state = [None, None]
                deferred.setdefault(step + 2, []).append(piece2)
                deferred.setdefault(step + 3, []).append(piece3)
                deferred.setdefault(step + 4, []).append(piece4)

            for u in range(NU):
                qc, pair, kc = unit(u)
                if pair == 0 and kc == 0:
                    # qc start: prefetch next mask, build filler queue
                    if qc + 1 < NQC:
                        mq_tiles[qc + 1] = prefetch_mask(qc + 1)
                    fillers = []
                    if qc + 1 < NQC:
                        xq = load_x_window(qc + 1, f"xq{qc + 1}")
                        fillers += qproj_fillers(qc + 1, xq)
                    if qc > 0:
                        fillers += outproj_fillers(qc - 1)
                    qc_state[qc] = {"fillers": fillers, "fi": 0}
                st = qc_state[qc]
                mq = mq_tiles[qc]

                ss = aps.tile([P, 2, 512], F32, tag="ss", bufs=2,
                              name=f"ss{qc}_{pair}_{kc}")
                for hp in range(2):
                    nc.tensor.matmul(
                        ss[:, hp, :],
                        lhsT=kT[hp * DH:(hp + 1) * DH, pair,
                                kc * 128:(kc + 1) * 128],
                        rhs=qT[hp * DH:(hp + 1) * DH, pair,
                               qc * 512:(qc + 1) * 512],
                        start=True, stop=True)
                sm = smp.tile([P, 2, 512], F32, tag="sm",
                              name=f"sm{qc}_{pair}_{kc}")
                nc.vector.tensor_tensor(sm, ss[:, :, :],
                                        mq[:, :, kc, :], OP.mult)
                ex = exp_pool.tile([P, 2, 512], F16, tag="ex",
                                   name=f"ex{qc}_{pair}_{kc}")
                nc.scalar.activation(out=ex, in_=sm, func=AF.Exp, scale=0.125)
                ex_tiles[u] = ex
                if u >= LAG:
                    emit_ctx(u - LAG, u)
                for fn in deferred.pop(u, ()):
                    fn()
                # Fillers only after the first 8 units of a qc: the previous
                # qc's last cpr tile is written by deferred normalize pieces
                # in the first ~4 steps, and reads emitted before those
                # writes would not be dependency-tracked.
                slot = (u % 64) >= 8 and (
                    u % 4 == 1 or ((u % 64) >= 40 and u % 4 == 3))
                if slot and st["fi"] < len(st["fillers"]):
                    st["fillers"][st["fi"]]()
                    st["fi"] += 1
                # drain leftover fillers at the very end of each qc
                if kc == 15 and pair == 3:
                    while st["fi"] < len(st["fillers"]):
                        st["fillers"][st["fi"]]()
                        st["fi"] += 1

            for i, u in enumerate(range(NU - LAG, NU)):
                step = NU + i
                emit_ctx(u, step)
                for fn in deferred.pop(step, ()):
                    fn()
            for step in sorted(deferred):
                for fn in deferred.pop(step):
                    fn()
            for el in outproj_fillers(NQC - 1):
                el()
    nc.compile()
    return nc


def _get_nc():
    if "nc" not in _CACHE:
        _CACHE["nc"] = _build()
    return _CACHE["nc"]


def kernel(input, mask, Wq, bq, Wk, bk, Wv, bv, Wo, bo):
    x = np.asarray(input, dtype=np.float32)
    m = np.asarray(mask, dtype=np.float32)
    Wq = np.asarray(Wq, dtype=np.float32)
    Wk = np.asarray(Wk, dtype=np.float32)
    Wv = np.asarray(Wv, dtype=np.float32)
    Wo = np.asarray(Wo, dtype=np.float32)
    bq = np.asarray(bq, dtype=np.float32)
    bk = np.asarray(bk, dtype=np.float32)
    bv = np.asarray(bv, dtype=np.float32)
    bo = np.asarray(bo, dtype=np.float32)
    B = x.shape[0]
    assert x.shape == (B, NSEQ, DMODEL) and B == 4

    nc = _get_nc()

    def pack_w(W):  # [1024, 512] -> [p, dc, m]
        return np.ascontiguousarray(
            W.reshape(DC, P, HD).transpose(1, 0, 2)).astype(np.float16)

    in_maps = []
    for b in range(B):
        xT = x[b].T.astype(np.float16)            # [1024, 2048]
        x_pre = np.ascontiguousarray(
            xT.reshape(DC, P, NQC, 512).transpose(2, 1, 0, 3))
        mT = m[b].T.astype(np.float16)            # [k, q]
        m_pre = np.ascontiguousarray(
            mT.reshape(16, P, NQC, 512).transpose(2, 1, 0, 3))
        for hg in range(2):
            sl = slice(hg * HD, (hg + 1) * HD)
            in_maps.append({
                "xp": x_pre,
                "maskp": m_pre,
                "wq": pack_w(Wq[:, sl]),
                "wk": pack_w(Wk[:, sl]),
                "wv": pack_w(Wv[:, sl]),
                "wo": np.ascontiguousarray(
                    Wo[sl].reshape(4, P, DMODEL).transpose(1, 0, 2)
                ).astype(np.float16),
                "ident": np.concatenate(
                    [np.zeros((DH, DH), np.float16),
                     np.eye(DH, dtype=np.float16)], axis=1),
                "bq2": np.ascontiguousarray(bq[sl].reshape(4, P).T),
                "bk2": np.ascontiguousarray(bk[sl].reshape(4, P).T),
                "bvr": np.ascontiguousarray(
                    np.broadcast_to(bv[sl], (P, HD))),
                "ones2": np.ones((P, 512), dtype=np.float32),
            })

    res = bass_utils.run_bass_kernel_spmd(nc, in_maps, core_ids=list(range(8)))
    global LAST_RESULTS
    LAST_RESULTS = res

    out = np.empty((B, NSEQ, DMODEL), dtype=np.float32)
    for b in range(B):
        out[b] = (res.results[2 * b]["y"].astype(np.float32)
                  + res.results[2 * b + 1]["y"].astype(np.float32) + bo)
    return out


# revision 38
# speedup vs baseline: 1.7541x; 1.0195x over previous
"""Multi-head attention (B=4, N=2048, D=1024, H=16) on 8 Trainium2 NeuronCores.

Sharding: core c = 2*b + hg handles batch b and head-group hg (8 of 16 heads).
Host pre-transposes x and mask per batch into DMA-friendly contiguous layouts,
slices Wq/Wk/Wv columns and Wo rows per head group, and sums the two partial
outputs per batch (+ bo) at the end.

v3 design notes:
  - The PE only reaches 2.4 GHz after ~3us of uninterrupted work, so the
    attention stream is software-pipelined: ctx matmuls are emitted LAG units
    behind their scores/mult/exp producers so the in-order PE queue never
    blocks on the DVE->ACT chain; per-pair softmax normalization is deferred
    into the next pair's unit stream.
  - q-projection for q-chunks 1..3 and the output projection are deferred and
    interleaved into the (elementwise-bound) attention stream as PE filler.
  - Mask is fp16 and host-packed [qc, p, kc, 512] so each DMA is one
    16KB-per-partition contiguous descriptor set; x and weights likewise.
  - Softmax: ctx matmul lhsT is v augmented with a ones column -> psum row 64
    accumulates the denominator. DVE reciprocal -> tiny PE ones-matmul
    broadcasts it across 64 partitions (GpSimd partition_broadcast has ~10us
    dispatch latency; PE does it in 0.2us) -> DVE multiplies, PE ident-shift
    packs the second head into rows 64..127.
  - PSUM: scores ring 2x[128,2,512] (4 banks) + ctx ring 2x[65,2,512] (4).
    Normalize/out-proj/q-proj psum shares the scores ring.
"""
import os
from contextlib import ExitStack

import numpy as np

from concourse import bacc, mybir, tile
from concourse import bass_utils

P = 128
NSEQ = 2048          # sequence length
DMODEL = 1024        # model dim
HD = 512             # per-core head dim total (8 heads x 64)
NH = 8               # heads per core
DH = 64              # head depth
DC = DMODEL // P     # 8 d_model chunks
NQC = 4              # q chunks of 512
F32 = mybir.dt.float32
F32R = mybir.dt.float32r
F16 = mybir.dt.float16
AF = mybir.ActivationFunctionType
OP = mybir.AluOpType

N_WARM = 12          # PE clock warm-up matmuls
FILLER_EVERY = 4     # emit one filler element every this many units
LAG = 3              # units between scores emission and ctx emission

_CACHE: dict = {}
LAST_RESULTS = None


def _patch_act_tables():
    """Force every activation onto the one table set containing
    exp+ln+copy+identity, so the kernel performs a single ACT_TABLE_LOAD
    instead of thrashing between per-function sets (1.3us each)."""
    import functools
    from concourse import bacc as _bacc
    from concourse import hw_specs as _hw
    if getattr(_bacc, "_act_tables_patched", False):
        return
    orig = _hw.get_activation_tables

    @functools.cache
    def patched(arch):
        tabs = dict(orig(arch))
        full = "natural_log_exp_and_others"
        keep = tabs[full]
        strip = {f for f in keep}
        out = {}
        for name, funcs in tabs.items():
            out[name] = funcs if name == full else (funcs - strip)
        return out

    _bacc.get_activation_tables = patched
    _bacc._act_tables_patched = True


def _build():
    _patch_act_tables()
    nc = bacc.Bacc("TRN2", target_bir_lowering=False, debug=False,
                   enable_asserts=False, num_devices=8)

    x_d = nc.dram_tensor("xp", [NQC, P, DC, 512], F16, kind="ExternalInput").ap()
    mask_d = nc.dram_tensor("maskp", [NQC, P, 16, 512], F16,
                            kind="ExternalInput").ap()
    wq_d = nc.dram_tensor("wq", [P, DC, 512], F16, kind="ExternalInput").ap()
    wk_d = nc.dram_tensor("wk", [P, DC, 512], F16, kind="ExternalInput").ap()
    wv_d = nc.dram_tensor("wv", [P, DC, 512], F16, kind="ExternalInput").ap()
    wo_d = nc.dram_tensor("wo", [P, 4, DMODEL], F16, kind="ExternalInput").ap()
    id_d = nc.dram_tensor("ident", [DH, P], F16, kind="ExternalInput").ap()
    bq_d = nc.dram_tensor("bq2", [P, 4], F32, kind="ExternalInput").ap()
    bk_d = nc.dram_tensor("bk2", [P, 4], F32, kind="ExternalInput").ap()
    bvr_d = nc.dram_tensor("bvr", [P, HD], F32, kind="ExternalInput").ap()
    ones_d = nc.dram_tensor("ones2", [P, 512], F32R, kind="ExternalInput").ap()
    y_d = nc.dram_tensor("y", [NSEQ, DMODEL], F16, kind="ExternalOutput").ap()

    with tile.TileContext(nc) as tc, ExitStack() as ctx:
        persist = ctx.enter_context(tc.tile_pool(name="persist", bufs=1))
        qT = persist.tile([P, 4, NSEQ], F16)     # [hd%128, hd-chunk, seq]
        kT = persist.tile([P, 4, NSEQ], F16)
        v = persist.tile([P, 16, NH, DH + 1], F16)  # [seq%128, seq-chunk, h, d|1]
        wq = persist.tile([P, DC, HD], F16)
        wk = persist.tile([P, DC, HD], F16)
        wv = persist.tile([P, DC, HD], F16)
        wo = persist.tile([P, 4, DMODEL], F16)
        ident = persist.tile([DH, P], F16)
        ones = persist.tile([P, 512], F32R)
        bqs = persist.tile([P, 4], F32)
        bks = persist.tile([P, 4], F32)
        bvr = persist.tile([P, HD], F32)

        mqp = ctx.enter_context(tc.tile_pool(name="mq", bufs=2))
        exp_pool = ctx.enter_context(tc.tile_pool(name="ex", bufs=4))
        smp = ctx.enter_context(tc.tile_pool(name="sm", bufs=3))
        ctxp = ctx.enter_context(tc.tile_pool(name="cpr", bufs=8))
        rtp = ctx.enter_context(tc.tile_pool(name="rt", bufs=1))
        rbsbp = ctx.enter_context(tc.tile_pool(name="rbsb", bufs=1))
        cxop = ctx.enter_context(tc.tile_pool(name="cxo", bufs=1))
        pyp = ctx.enter_context(tc.tile_pool(name="py_sb", bufs=2))
        xwp = ctx.enter_context(tc.tile_pool(name="xw", bufs=2))

        def load_x_window(n, name):
            xw = xwp.tile([P, DC, 512], F16, tag="xw", name=name)
            nc.sync.dma_start(out=xw, in_=x_d[n])
            return xw

        # ---- initial DMAs (queue order matters: earliest-needed first) ----
        nc.sync.dma_start(out=ones, in_=ones_d)
        xw0 = load_x_window(0, "xn0")
        nc.sync.dma_start(out=wk, in_=wk_d)
        nc.sync.dma_start(out=wv, in_=wv_d)
        nc.sync.dma_start(out=wq, in_=wq_d)
        nc.sync.dma_start(out=bqs, in_=bq_d)
        nc.sync.dma_start(out=bks, in_=bk_d)
        nc.sync.dma_start(out=bvr, in_=bvr_d)
        nc.sync.dma_start(out=ident, in_=id_d)
        nc.sync.dma_start(out=wo, in_=wo_d)

        nc.vector.tensor_copy(
            out=v[:, :, :, DH],
            in_=ones[:, 0:P].rearrange("p (a b) -> p a b", b=NH))

        def prefetch_mask(qc):
            mq = mqp.tile([P, 2, 16, 512], F16, tag="mq", name=f"mq{qc}")
            for hp in range(2):
                nc.sync.dma_start(out=mq[:, hp, :, :], in_=mask_d[qc])
            return mq

        # ---------------- phase P: projections ----------------
        with tc.tile_pool(name="pp", bufs=6, space="PSUM") as pp:
            wt = pp.tile([P, 512], F32, tag="pp", name="warm")
            for i in range(N_WARM):
                nc.tensor.matmul(wt, lhsT=ones[:, 0:P], rhs=ones,
                                 start=(i == 0), stop=(i == N_WARM - 1))
            mq_tiles = {0: prefetch_mask(0)}
            for n in range(NQC):
                x_n = xw0 if n == 0 else load_x_window(n, f"xn{n}")
                for m in range(4):
                    ps = pp.tile([P, 512], F32, tag="pp", name="psk")
                    for dc in range(DC):
                        nc.tensor.matmul(ps,
                                         lhsT=wk[:, dc, m * 128:(m + 1) * 128],
                                         rhs=x_n[:, dc, :],
                                         start=(dc == 0), stop=(dc == DC - 1))
                    nc.scalar.activation(
                        out=kT[:, m, n * 512:(n + 1) * 512], in_=ps,
                        func=AF.Identity, bias=bks[:, m:m + 1], scale=1.0)
                for s4 in range(4):
                    psv = pp.tile([P, 512], F32, tag="pp", name="psv")
                    for dc in range(DC):
                        nc.tensor.matmul(psv,
                                         lhsT=x_n[:, dc, s4 * 128:(s4 + 1) * 128],
                                         rhs=wv[:, dc, :],
                                         start=(dc == 0), stop=(dc == DC - 1))
                    s = n * 4 + s4
                    nc.vector.tensor_tensor(
                        v[:, s, :, 0:DH],
                        psv.rearrange("p (h d) -> p h d", d=DH),
                        bvr.rearrange("p (h d) -> p h d", d=DH),
                        OP.add)
                if n == 0:
                    for m in range(4):
                        psq = pp.tile([P, 512], F32, tag="pp", name="psq0")
                        for dc in range(DC):
                            nc.tensor.matmul(psq,
                                             lhsT=wq[:, dc, m * 128:(m + 1) * 128],
                                             rhs=x_n[:, dc, :],
                                             start=(dc == 0), stop=(dc == DC - 1))
                        nc.scalar.activation(
                            out=qT[:, m, 0:512], in_=psq,
                            func=AF.Identity, bias=bqs[:, m:m + 1], scale=1.0)

        # ---------------- attention + output projection ----------------
        ctx_tiles = {qc: [] for qc in range(NQC)}
        with tc.tile_pool(name="aps", bufs=1, space="PSUM") as aps:

            def qproj_fillers(qc, xq):
                els = []
                for m in range(4):
                    state = {}

                    def el1(m=m, state=state, xq=xq):
                        psq = aps.tile([P, 2, 512], F32, tag="ss",
                                       bufs=3, name=f"psq{qc}_{m}")
                        state["psq"] = psq
                        for dc in range(4):
                            nc.tensor.matmul(
                                psq[:, 0, :],
                                lhsT=wq[:, dc, m * 128:(m + 1) * 128],
                                rhs=xq[:, dc, :],
                                start=(dc == 0), stop=False)

                    def el2(m=m, state=state, qc=qc, xq=xq):
                        psq = state["psq"]
                        for dc in range(4, DC):
                            nc.tensor.matmul(
                                psq[:, 0, :],
                                lhsT=wq[:, dc, m * 128:(m + 1) * 128],
                                rhs=xq[:, dc, :],
                                start=False, stop=(dc == DC - 1))
                        nc.scalar.activation(
                            out=qT[:, m, qc * 512:(qc + 1) * 512],
                            in_=psq[:, 0, :],
                            func=AF.Identity, bias=bqs[:, m:m + 1], scale=1.0)

                    els += [el1, el2]
                return els

            def outproj_fillers(qc):
                els = []
                for qs in range(4):
                    state = {}

                    def el1(qs=qs, state=state, qc=qc):
                        py = aps.tile([P, 2, 512], F32, tag="ss",
                                      bufs=3, name=f"py{qc}_{qs}")
                        state["py"] = py
                        for c in range(4):
                            nc.tensor.matmul(
                                py[:, 0, :],
                                lhsT=ctx_tiles[qc][c][:, qs * 128:(qs + 1) * 128],
                                rhs=wo[:, c, 0:512],
                                start=(c == 0), stop=(c == 3))

                    def el2(qs=qs, state=state, qc=qc):
                        py = state["py"]
                        for c in range(4):
                            nc.tensor.matmul(
                                py[:, 1, :],
                                lhsT=ctx_tiles[qc][c][:, qs * 128:(qs + 1) * 128],
                                rhs=wo[:, c, 512:1024],
                                start=(c == 0), stop=(c == 3))
                        py_sb = pyp.tile([P, 2, 512], F16, tag="py",
                                         name=f"ysb{qc}_{qs}")
                        nc.scalar.activation(out=py_sb, in_=py, func=AF.Copy)
                        r0 = qc * 512 + qs * 128
                        nc.sync.dma_start(out=y_d[r0:r0 + 128, :],
                                          in_=py_sb.rearrange("p a b -> p (a b)"))

                    els += [el1, el2]
                return els

            # ---- software-pipelined unit stream ----
            NU = NQC * 4 * 16  # 256 units
            ex_tiles = {}      # u -> exp tile
            pc_tiles = {}      # (qc, pair) -> ctx psum accumulator
            deferred = {}      # step -> list of closures (normalize pieces)
            qc_state = {}

            def unit(u):
                return u // 64, (u // 16) % 4, u % 16  # qc, pair, kc

            def emit_ctx(u, step):
                qc, pair, kc = unit(u)
                key = (qc, pair)
                if key not in pc_tiles:
                    pc_tiles[key] = aps.tile([DH + 1, 2, 512], F32, tag="pc",
                                             bufs=1, name=f"pc{qc}_{pair}")
                pc = pc_tiles[key]
                ex = ex_tiles.pop(u)
                for hp in range(2):
                    nc.tensor.matmul(
                        pc[:, hp, :],
                        lhsT=v[:, kc, 2 * pair + hp, :],
                        rhs=ex[:, hp, :],
                        start=(kc == 0), stop=(kc == 15))
                if kc == 15:
                    schedule_normalize(step, qc, pair, pc)

            def schedule_normalize(step, qc, pair, pc):
                """Emit recip now; broadcast+mults and shift at later steps."""
                dd = rtp.tile([1, 2, 512], F32R, tag="rt",
                              name=f"dd{qc}_{pair}")
                nc.scalar.activation(out=dd, in_=pc[DH:DH + 1, :, :],
                                     func=AF.Copy)
                cpr = ctxp.tile([P, 512], F16, tag="cpr",
                                name=f"cpr{qc}_{pair}")
                ctx_tiles[qc].append(cpr)

                def piece2():
                    rb = aps.tile([P, 2, 512], F32, tag="ss", bufs=3,
                                  name=f"rb{qc}_{pair}")
                    for hp in range(2):
                        nc.tensor.matmul(rb[0:DH, hp, :],
                                         lhsT=ones[0:1, 0:DH],
                                         rhs=dd[0:1, hp, :],
                                         start=True, stop=True)
                    rb_sb = rbsbp.tile([DH, 2, 512], F32, tag="rb",
                                       name=f"rbsb{qc}_{pair}")
                    state[0] = rb_sb
                    nc.vector.reciprocal_approx_fast(rb_sb, rb[0:DH, :, :])

                def piece3():
                    rb_sb = state[0]
                    nc.vector.tensor_tensor(cpr[0:DH, :], pc[0:DH, 0, :],
                                            rb_sb[:, 0, :], OP.mult)
                    cxo = cxop.tile([DH, 512], F16, tag="cxo",
                                    name=f"cxo{qc}_{pair}")
                    state[1] = cxo
                    nc.vector.tensor_tensor(cxo, pc[0:DH, 1, :],
                                            rb_sb[:, 1, :], OP.mult)

                def piece4():
                    cxo = state[1]
                    sh = aps.tile([P, 2, 512], F32, tag="ss", bufs=3,
                                  name=f"sh{qc}_{pair}")
                    nc.tensor.matmul(sh[:, 0, :], lhsT=ident, rhs=cxo,
                                     start=True, stop=True)
                    nc.scalar.activation(out=cpr[DH:P, :],
                                         in_=sh[DH:P, 0, :], func=AF.Copy)

                state = [None, None]
                deferred.setdefault(step + 1, []).append(piece2)
                deferred.setdefault(step + 2, []).append(piece3)
                deferred.setdefault(step + 3, []).append(piece4)

            for u in range(NU):
                qc, pair, kc = unit(u)
                if pair == 0 and kc == 0:
                    # qc start: prefetch next mask, build filler queue
                    if qc + 1 < NQC:
                        mq_tiles[qc + 1] = prefetch_mask(qc + 1)
                    fillers = []
                    if qc + 1 < NQC:
                        xq = load_x_window(qc + 1, f"xq{qc + 1}")
                        fillers += qproj_fillers(qc + 1, xq)
                    if qc > 0:
                        fillers += outproj_fillers(qc - 1)
                    qc_state[qc] = {"fillers": fillers, "fi": 0}
                st = qc_state[qc]
                mq = mq_tiles[qc]

                ss = aps.tile([P, 2, 512], F32, tag="ss", bufs=3,
                              name=f"ss{qc}_{pair}_{kc}")
                for hp in range(2):
                    nc.tensor.matmul(
                        ss[:, hp, :],
                        lhsT=kT[hp * DH:(hp + 1) * DH, pair,
                                kc * 128:(kc + 1) * 128],
                        rhs=qT[hp * DH:(hp + 1) * DH, pair,
                               qc * 512:(qc + 1) * 512],
                        start=True, stop=True)
                sm = smp.tile([P, 2, 512], F16, tag="sm",
                              name=f"sm{qc}_{pair}_{kc}")
                nc.vector.tensor_tensor(sm, ss[:, :, :],
                                        mq[:, :, kc, :], OP.mult)
                ex = exp_pool.tile([P, 2, 512], F16, tag="ex",
                                   name=f"ex{qc}_{pair}_{kc}")
                nc.scalar.activation(out=ex, in_=sm, func=AF.Exp, scale=0.125)
                ex_tiles[u] = ex
                if u >= LAG:
                    emit_ctx(u - LAG, u)
                for fn in deferred.pop(u, ()):
                    fn()
                # Fillers only after the first 8 units of a qc: the previous
                # qc's last cpr tile is written by deferred normalize pieces
                # in the first ~4 steps, and reads emitted before those
                # writes would not be dependency-tracked.
                slot = (u % 64) >= 8 and (
                    u % 4 == 1 or ((u % 64) >= 40 and u % 4 == 3))
                if slot and st["fi"] < len(st["fillers"]):
                    st["fillers"][st["fi"]]()
                    st["fi"] += 1
                # drain leftover fillers at the very end of each qc
                if kc == 15 and pair == 3:
                    while st["fi"] < len(st["fillers"]):
                        st["fillers"][st["fi"]]()
                        st["fi"] += 1

            for i, u in enumerate(range(NU - LAG, NU)):
                step = NU + i
                emit_ctx(u, step)
                for fn in deferred.pop(step, ()):
                    fn()
            for step in sorted(deferred):
                for fn in deferred.pop(step):
                    fn()
            for el in outproj_fillers(NQC - 1):
                el()
    nc.compile()
    return nc


def _get_nc():
    if "nc" not in _CACHE:
        _CACHE["nc"] = _build()
    return _CACHE["nc"]


def kernel(input, mask, Wq, bq, Wk, bk, Wv, bv, Wo, bo):
    x = np.asarray(input, dtype=np.float32)
    m = np.asarray(mask, dtype=np.float32)
    Wq = np.asarray(Wq, dtype=np.float32)
    Wk = np.asarray(Wk, dtype=np.float32)
    Wv = np.asarray(Wv, dtype=np.float32)
    Wo = np.asarray(Wo, dtype=np.float32)
    bq = np.asarray(bq, dtype=np.float32)
    bk = np.asarray(bk, dtype=np.float32)
    bv = np.asarray(bv, dtype=np.float32)
    bo = np.asarray(bo, dtype=np.float32)
    B = x.shape[0]
    assert x.shape == (B, NSEQ, DMODEL) and B == 4

    nc = _get_nc()

    def pack_w(W):  # [1024, 512] -> [p, dc, m]
        return np.ascontiguousarray(
            W.reshape(DC, P, HD).transpose(1, 0, 2)).astype(np.float16)

    in_maps = []
    for b in range(B):
        xT = x[b].T.astype(np.float16)            # [1024, 2048]
        x_pre = np.ascontiguousarray(
            xT.reshape(DC, P, NQC, 512).transpose(2, 1, 0, 3))
        mT = m[b].T.astype(np.float16)            # [k, q]
        m_pre = np.ascontiguousarray(
            mT.reshape(16, P, NQC, 512).transpose(2, 1, 0, 3))
        for hg in range(2):
            sl = slice(hg * HD, (hg + 1) * HD)
            in_maps.append({
                "xp": x_pre,
                "maskp": m_pre,
                "wq": pack_w(Wq[:, sl]),
                "wk": pack_w(Wk[:, sl]),
                "wv": pack_w(Wv[:, sl]),
                "wo": np.ascontiguousarray(
                    Wo[sl].reshape(4, P, DMODEL).transpose(1, 0, 2)
                ).astype(np.float16),
                "ident": np.concatenate(
                    [np.zeros((DH, DH), np.float16),
                     np.eye(DH, dtype=np.float16)], axis=1),
                "bq2": np.ascontiguousarray(bq[sl].reshape(4, P).T),
                "bk2": np.ascontiguousarray(bk[sl].reshape(4, P).T),
                "bvr": np.ascontiguousarray(
                    np.broadcast_to(bv[sl], (P, HD))),
                "ones2": np.ones((P, 512), dtype=np.float32),
            })

    res = bass_utils.run_bass_kernel_spmd(nc, in_maps, core_ids=list(range(8)))
    global LAST_RESULTS
    LAST_RESULTS = res

    out = np.empty((B, NSEQ, DMODEL), dtype=np.float32)
    for b in range(B):
        out[b] = (res.results[2 * b]["y"].astype(np.float32)
                  + res.results[2 * b + 1]["y"].astype(np.float32) + bo)
    return out
